# revision 1
# baseline (speedup 1.0000x reference)
"""Self-contained Trainium2 attention-block kernel (8 NeuronCores, SPMD).

Problem: x[4,4096,128], Wq/Wk[64,128], Wv[128,128] ->
  softmax((x Wq^T)(x Wk^T)^T / 8) (x Wv^T)   -> [4,4096,128] f32

Sharding: data-parallel over batch (4) x query-halves (2) = 8 cores.
Each core: q rows 2048, full K/V (4096) recomputed locally. No collectives.

Per-core pipeline (all matmuls bf16):
  scores transposed ST[k,q] = KT_chunk.T @ QT (row-tiled 64-contraction
  pairs run concurrently); exp on ScalarE with fused 1/8 scale (no
  max-subtract: scores ~N(0,1)); PV accumulated as outT[v,q] over 32
  k-chunks; softmax denominator via bf16 add-tree (DVE + some GpSimd);
  DMA-transposes flip outT and D partials back to [q,...]; final
  per-partition 1/D scale; bf16 DRAM out upconverted on host.
ST emission runs two groups ahead of PV so the in-order PE never waits
on the exp of the current group.
"""

import sys

sys.path.insert(0, "/opt/trn_rl_repo")

from contextlib import ExitStack

import ml_dtypes
import numpy as np

import concourse.bass as bass  # noqa: F401
import concourse.bacc as bacc
import concourse.tile as tile
from concourse import mybir
from concourse.bass_utils import run_bass_kernel_spmd

BF16 = mybir.dt.bfloat16
F32 = mybir.dt.float32
NPBF16 = ml_dtypes.bfloat16

B, S, D, A = 4, 4096, 128, 64
NQ = S // 2          # q rows per core
QB = 512             # q block (psum bank free size)
KC = 128             # k chunk (matmul contraction tile)
NKC = S // KC        # 32 chunks
NQB = NQ // QB       # 4 q blocks
GROUP = 2            # k chunks per exp group ([128,1024] psum tile)
NGRP = NKC // GROUP  # 16 groups per block
EXP = mybir.ActivationFunctionType.Exp
AXX = mybir.AxisListType.X

_CACHED_NC = None


def _log(msg):
    import time as _t
    print(f"[kernel {_t.strftime('%H:%M:%S')}] {msg}", file=sys.stderr, flush=True)


def build_nc():
    _log("build_nc: tracing graph")
    nc = bacc.Bacc(
        "TRN2", target_bir_lowering=False, debug=False,
        enable_asserts=False, num_devices=8,
    )
    xT = nc.dram_tensor("xT", [D, S], BF16, kind="ExternalInput").ap()
    xqT = nc.dram_tensor("xqT", [D, NQ], BF16, kind="ExternalInput").ap()
    wqTd = nc.dram_tensor("wqTd", [D, 128], BF16, kind="ExternalInput").ap()
    wkTd = nc.dram_tensor("wkTd", [D, 128], BF16, kind="ExternalInput").ap()
    wvT = nc.dram_tensor("wvT", [D, D], BF16, kind="ExternalInput").ap()
    ones = nc.dram_tensor("ones", [D, D], BF16, kind="ExternalInput").ap()
    # outT layout [v, q]; host transposes for free during gather
    out = nc.dram_tensor("out", [D, NQ], BF16, kind="ExternalOutput").ap()

    with tile.TileContext(nc) as tc, ExitStack() as ctx:
        persist = ctx.enter_context(tc.tile_pool(name="persist", bufs=1))
        # PSUM: st 2x(2 banks) + pv 2x(1) + proj 2x(1) = 8 banks
        ps_st = ctx.enter_context(tc.tile_pool(name="ps_st", bufs=2, space="PSUM"))
        ps_pv = ctx.enter_context(tc.tile_pool(name="ps_pv", bufs=2, space="PSUM"))
        ps_pj = ctx.enter_context(tc.tile_pool(name="ps_pj", bufs=2, space="PSUM"))
        ppool = ctx.enter_context(tc.tile_pool(name="ppool", bufs=4))
        tpool = ctx.enter_context(tc.tile_pool(name="tpool", bufs=8))
        mpool = ctx.enter_context(tc.tile_pool(name="mpool", bufs=4))

        # ---- persistent SBUF + input DMAs ----
        xqT_s = persist.tile([D, NQ], BF16, tag="xqT_s")
        nc.sync.dma_start(xqT_s[:, 0:QB], xqT[:, 0:QB])
        xT_s = persist.tile([D, S], BF16, tag="xT_s")
        for j in range(4):  # split so proj matmuls unblock progressively
            nc.sync.dma_start(xT_s[:, j * 1024:(j + 1) * 1024],
                              xT[:, j * 1024:(j + 1) * 1024])
        nc.sync.dma_start(xqT_s[:, QB:], xqT[:, QB:])
        wq_s = persist.tile([D, 128], BF16, tag="wq_s")
        nc.sync.dma_start(wq_s[:], wqTd[:])
        wk_s = persist.tile([D, 128], BF16, tag="wk_s")
        nc.sync.dma_start(wk_s[:], wkTd[:])
        wv_s = persist.tile([D, D], BF16, tag="wv_s")
        nc.sync.dma_start(wv_s[:], wvT[:])
        ones_s = persist.tile([D, D], BF16, tag="ones_s")
        nc.sync.dma_start(ones_s[:], ones[:])

        KT_s = persist.tile([128, S], BF16, tag="KT_s")   # duplicated halves
        QT_s = persist.tile([128, NQ], BF16, tag="QT_s")  # duplicated halves
        V_s = persist.tile([128, S], BF16, tag="V_s")     # [:,c*128:+128]=V[c*128:+128,:]

        # prewarm the exp table so the first real exp doesn't eat the
        # ~2.7us ACT_TABLE_LOAD on the critical path
        warm = persist.tile([1, 1], F32, tag="warm")
        nc.gpsimd.memset(warm[:], 1.0)
        warm2 = persist.tile([1, 1], F32, tag="warm2")
        nc.scalar.activation(warm2[:], warm[:], EXP)

        # ---- projections ----
        # early (ACT copies): QT block0 + KT j0 unblock the first STs fast
        def proj_mm(dst, w, src_slice, copy_engine):
            pt = ps_pj.tile([128, QB], F32, tag="pj")
            nc.tensor.matmul(pt[:], w, src_slice, start=True, stop=True)
            copy_engine(dst, pt[:])

        act_cp = nc.scalar.copy
        dve_cp = nc.vector.tensor_copy
        proj_mm(QT_s[:, 0:QB], wq_s[:], xqT_s[:, 0:QB], act_cp)
        proj_mm(KT_s[:, 0:QB], wk_s[:], xT_s[:, 0:QB], act_cp)
        proj_mm(KT_s[:, QB:2 * QB], wk_s[:], xT_s[:, QB:2 * QB], act_cp)
        # V projection: 4 chunks of [s=128, v=128] per psum tile
        for g in range(8):
            pt = ps_pj.tile([128, QB], F32, tag="pj")
            for i in range(4):
                c = g * 4 + i
                nc.tensor.matmul(pt[:, i * 128:(i + 1) * 128],
                                 xT_s[:, c * 128:(c + 1) * 128], wv_s[:],
                                 start=True, stop=True)
            dve_cp(V_s[:, g * QB:(g + 1) * QB], pt[:])
        for j in range(2, S // QB):
            proj_mm(KT_s[:, j * QB:(j + 1) * QB], wk_s[:],
                    xT_s[:, j * QB:(j + 1) * QB], dve_cp)
        for j in range(1, NQ // QB):
            proj_mm(QT_s[:, j * QB:(j + 1) * QB], wq_s[:],
                    xqT_s[:, j * QB:(j + 1) * QB], dve_cp)

        # ---- attention: flat software pipeline over (qblock, group) ----
        ALL = [(qb, g) for qb in range(NQB) for g in range(NGRP)]

        def emit_st(qb, g):
            q0 = qb * QB
            st = ps_st.tile([128, GROUP * QB], F32, tag="st")
            for i in range(GROUP):
                kc = g * GROUP + i
                h = kc % 2  # row-tile half: concurrent 64-contraction pairs
                lhsT = KT_s[h * 64:(h + 1) * 64, kc * KC:(kc + 1) * KC]
                rhs = QT_s[h * 64:(h + 1) * 64, q0:q0 + QB]
                nc.tensor.matmul(st[:, i * QB:(i + 1) * QB], lhsT, rhs,
                                 start=True, stop=True)
            return st

        st_tiles = {}
        st_tiles[ALL[0]] = emit_st(*ALL[0])
        st_tiles[ALL[1]] = emit_st(*ALL[1])

        pv_tiles = {}
        stacks = {qb: [] for qb in range(NQB)}  # binary-counter D trees
        nadd = {qb: 0 for qb in range(NQB)}

        def tree_add(qb, dst, a, b_):
            # route a fraction of adds to the otherwise-idle GpSimd
            eng = nc.gpsimd if nadd[qb] % 4 == 3 else nc.vector
            eng.tensor_add(dst, a, b_)
            nadd[qb] += 1

        def push(qb, t, lvl):
            st_ = stacks[qb]
            while st_ and st_[-1][0] == lvl:
                plvl, pt_ = st_.pop()
                nt = tpool.tile([128, QB], BF16, tag="tr")
                tree_add(qb, nt[:], pt_[:], t[:])
                t, lvl = nt, plvl + 1
            st_.append((lvl, t))

        def finish_block(qb):
            q0 = qb * QB
            # collapse D tree
            st_ = stacks[qb]
            while len(st_) > 1:
                l0, t0 = st_.pop()
                l1, t_1 = st_.pop()
                nt = tpool.tile([128, QB], BF16, tag="tr")
                tree_add(qb, nt[:], t_1[:], t0[:])
                st_.append((max(l0, l1) + 1, nt))
            s_tile = st_[0][1]  # [k-lane, q] bf16 chunk-sum

            # D[q] via ones-matmul: every output row = sum over k-lanes,
            # so the result arrives already partition-broadcast.
            dps = ps_pj.tile([128, QB], F32, tag="pj", name=f"dps{qb}")
            nc.tensor.matmul(dps[:], ones_s[:], s_tile[:], start=True, stop=True)
            dinvb = mpool.tile([128, QB], F32, tag="dinvb")
            nc.vector.reciprocal_approx_fast(dinvb[:], dps[:])
            outf = mpool.tile([128, QB], BF16, tag="outf")
            nc.vector.tensor_mul(outf[:], pv_tiles[qb][:], dinvb[:])
            nc.sync.dma_start(out[:, q0:q0 + QB], outf[:])
            del pv_tiles[qb]

        FINISH_DELAY = 3  # groups of the next block emitted before a finish
        for idx, (qb, g) in enumerate(ALL):
            st = st_tiles.pop((qb, g))
            p = ppool.tile([128, GROUP * QB], BF16, tag="p")
            nc.scalar.activation(p[:], st[:], EXP, scale=0.125)
            if qb not in pv_tiles:
                pv_tiles[qb] = ps_pv.tile([128, QB], F32, tag="pv",
                                          name=f"pv{qb}")
            pv = pv_tiles[qb]
            for i in range(GROUP):
                kc = g * GROUP + i
                nc.tensor.matmul(pv[:], V_s[:, kc * KC:(kc + 1) * KC],
                                 p[:, i * QB:(i + 1) * QB],
                                 start=(kc == 0), stop=(kc == NKC - 1))
            if idx + 2 < len(ALL):
                st_tiles[ALL[idx + 2]] = emit_st(*ALL[idx + 2])
            t1 = tpool.tile([128, QB], BF16, tag="tr")
            tree_add(qb, t1[:], p[:, 0:QB], p[:, QB:2 * QB])
            push(qb, t1, 1)
            if g == FINISH_DELAY - 1 and qb > 0:
                finish_block(qb - 1)
        finish_block(NQB - 1)

    _log("build_nc: bacc compile")
    nc.compile()
    _log("build_nc: done")
    return nc


def _host_prep(x, Wq, Wk, Wv):
    x = np.asarray(x, dtype=np.float32)
    Wq = np.asarray(Wq, dtype=np.float32)
    Wk = np.asarray(Wk, dtype=np.float32)
    Wv = np.asarray(Wv, dtype=np.float32)
    wqTd = np.ascontiguousarray(
        np.concatenate([Wq.T, Wq.T], axis=1)).astype(NPBF16)
    wkTd = np.ascontiguousarray(
        np.concatenate([Wk.T, Wk.T], axis=1)).astype(NPBF16)
    wvT = np.ascontiguousarray(Wv.T).astype(NPBF16)
    ones = np.ones((D, D), dtype=NPBF16)
    in_maps = []
    for c in range(8):
        b, h = c // 2, c % 2
        in_maps.append({
            "xT": np.ascontiguousarray(x[b].T).astype(NPBF16),
            "xqT": np.ascontiguousarray(
                x[b, h * NQ:(h + 1) * NQ].T).astype(NPBF16),
            "wqTd": wqTd, "wkTd": wkTd, "wvT": wvT, "ones": ones,
        })
    return in_maps


def run(x, Wq, Wk, Wv, trace=False, **kw):
    global _CACHED_NC
    if _CACHED_NC is None:
        _CACHED_NC = build_nc()
    in_maps = _host_prep(x, Wq, Wk, Wv)
    _log("run_bass_kernel_spmd (includes NEFF compile on first call)")
    res = run_bass_kernel_spmd(
        _CACHED_NC, in_maps, core_ids=list(range(8)), trace=trace, **kw)
    _log("run_bass_kernel_spmd returned")
    full = np.zeros((B, S, D), np.float32)
    for c in range(8):
        b, h = c // 2, c % 2
        full[b, h * NQ:(h + 1) * NQ] = np.asarray(
            res.results[c]["out"]).astype(np.float32).T
    return full, res


def kernel(x, Wq, Wk, Wv):
    full, _ = run(x, Wq, Wk, Wv, trace=False)
    return full



# revision 4
# speedup vs baseline: 1.3008x; 1.3008x over previous
"""Self-contained Trainium2 attention-block kernel (8 NeuronCores, SPMD).

Problem: x[4,4096,128], Wq/Wk[64,128], Wv[128,128] ->
  softmax((x Wq^T)(x Wk^T)^T / 8) (x Wv^T)   -> [4,4096,128] f32

Sharding: data-parallel over batch (4) x query-halves (2) = 8 cores.
Each core: q rows 2048, full K (4096) recomputed locally. No collectives.

v2 design (vs v1 which was ScalarE-bound at ~117us):
  - P*V reassociated as (P*X)*Wv^T: the per-chunk PV matmul uses raw x
    chunks as stationary weights (no V projection, no V casts); one
    [d,v] Wv matmul per q-block at the end; output DMA'd f32 from PSUM.
  - softmax denominator D via PE: accumulating ones-matmuls into a psum
    bank (rhs = bf16 pair-sums t1 for the first TJ groups, raw p chunks
    for the rest) instead of a deep DVE add-tree.
  - exp split across engines: every 4th group's exp runs on the DVE as
    a one-instruction Schraudolph bit-trick (u16 = s*A + B viewed as
    bf16 ~= exp(s/8)); the rest stay on ScalarE's table exp.
  - PE warm-up spin during the prologue so HAM reaches K=8/8 before
    steady state, plus weights-first DMA order and just-in-time
    projection emission.
"""

import sys

sys.path.insert(0, "/opt/trn_rl_repo")

from contextlib import ExitStack

import ml_dtypes
import numpy as np

import concourse.bass as bass  # noqa: F401
import concourse.bacc as bacc
import concourse.tile as tile
from concourse import mybir
from concourse.bass_utils import run_bass_kernel_spmd

BF16 = mybir.dt.bfloat16
F32 = mybir.dt.float32
U16 = mybir.dt.uint16
NPBF16 = ml_dtypes.bfloat16

B, S, D, A = 4, 4096, 128, 64
NQ = S // 2          # q rows per core
QB = 512             # q block (psum bank free size)
KC = 128             # k chunk (matmul contraction tile)
NKC = S // KC        # 32 chunks
NQB = NQ // QB       # 4 q blocks
GROUP = 2            # k chunks per exp group ([128,1024] psum tile)
NGRP = NKC // GROUP  # 16 groups per block
EXP = mybir.ActivationFunctionType.Exp

# tuning knobs
TJ = 8            # groups per qblock whose chunk-pair is pre-summed (t1) on DVE
DVE_EXP_MOD = 4   # groups with g % DVE_EXP_MOD == 1 run exp on DVE (Schraudolph)
DMM_LAG = 2       # groups between exp(g) and its D-matmuls (keeps PE fed)
FINISH_DELAY = 2  # groups into the next block before finishing a block
N_WARM = 14       # prologue PE warm-up matmuls (N=512)

# Schraudolph constants: u16 = round(s * SCH_A + SCH_B) viewed as bf16
# approximates exp(s/8).  t = s*log2(e)/8; bits = 128*t + (127*128 - C).
SCH_A = 128 * np.log2(np.e) / 8          # 23.083120654223414
SCH_B = 16256.0 - 7.5                    # C=7.5 splits round/trunc modes

_CACHED_NC = None


def _log(msg):
    import time as _t
    print(f"[kernel {_t.strftime('%H:%M:%S')}] {msg}", file=sys.stderr, flush=True)


def build_nc():
    _log("build_nc: tracing graph")
    nc = bacc.Bacc(
        "TRN2", target_bir_lowering=False, debug=False,
        enable_asserts=False, num_devices=8,
    )
    xT = nc.dram_tensor("xT", [D, S], BF16, kind="ExternalInput").ap()
    xc = nc.dram_tensor("xc", [128, S], BF16, kind="ExternalInput").ap()
    xqT = nc.dram_tensor("xqT", [D, NQ], BF16, kind="ExternalInput").ap()
    wqTd = nc.dram_tensor("wqTd", [D, 128], BF16, kind="ExternalInput").ap()
    wkTd = nc.dram_tensor("wkTd", [D, 128], BF16, kind="ExternalInput").ap()
    wvT = nc.dram_tensor("wvT", [D, D], BF16, kind="ExternalInput").ap()
    # outT layout [v, q] f32; host transposes during gather
    out = nc.dram_tensor("out", [D, NQ], F32, kind="ExternalOutput").ap()

    with tile.TileContext(nc) as tc, ExitStack() as ctx:
        persist = ctx.enter_context(tc.tile_pool(name="persist", bufs=1))
        # PSUM: st 2x(2 banks) + px 2x(1 bank) + misc 4x(1 bank shared pairwise)
        ps_st = ctx.enter_context(tc.tile_pool(name="ps_st", bufs=2, space="PSUM"))
        ps_px = ctx.enter_context(tc.tile_pool(name="ps_px", bufs=2, space="PSUM"))
        ps_ms = ctx.enter_context(tc.tile_pool(name="ps_ms", bufs=1, space="PSUM"))
        ppool = ctx.enter_context(tc.tile_pool(name="ppool", bufs=5))
        tpool = ctx.enter_context(tc.tile_pool(name="tpool", bufs=6))
        mpool = ctx.enter_context(tc.tile_pool(name="mpool", bufs=4))

        # ---- persistent SBUF ----
        wq_s = persist.tile([D, 128], BF16, tag="wq_s")
        wk_s = persist.tile([D, 128], BF16, tag="wk_s")
        wv_s = persist.tile([D, D], BF16, tag="wv_s")
        ones_s = persist.tile([128, 128], BF16, tag="ones_s")
        xqT_s = persist.tile([D, NQ], BF16, tag="xqT_s")
        xT_s = persist.tile([D, S], BF16, tag="xT_s")
        xc_s = persist.tile([128, S], BF16, tag="xc_s")
        KT_s = persist.tile([128, S], BF16, tag="KT_s")   # duplicated halves
        QT_s = persist.tile([128, NQ], BF16, tag="QT_s")  # duplicated halves

        # ones for the D-matmuls needs no DMA
        nc.gpsimd.memset(ones_s[:], 1.0)

        # input DMAs: tiny weights first, then the chunks the first
        # projections need, then the rest
        nc.sync.dma_start(wq_s[:], wqTd[:])
        nc.sync.dma_start(wk_s[:], wkTd[:])
        nc.sync.dma_start(xqT_s[:, 0:QB], xqT[:, 0:QB])
        for j in range(4):
            nc.sync.dma_start(xT_s[:, j * 1024:(j + 1) * 1024],
                              xT[:, j * 1024:(j + 1) * 1024])
        for j in range(4):
            nc.sync.dma_start(xc_s[:, j * 1024:(j + 1) * 1024],
                              xc[:, j * 1024:(j + 1) * 1024])
        nc.sync.dma_start(xqT_s[:, QB:], xqT[:, QB:])
        nc.sync.dma_start(wv_s[:], wvT[:])

        # prewarm the exp table (ScalarE) off the critical path
        warm = persist.tile([1, 1], F32, tag="warm")
        nc.gpsimd.memset(warm[:], 1.0)
        warm2 = persist.tile([1, 1], F32, tag="warm2")
        nc.scalar.activation(warm2[:], warm[:], EXP)

        # PE warm-up spin: keep TensorE busy through the prologue so HAM
        # reaches K=8/8 before the real matmuls start
        wsrc = persist.tile([128, QB], BF16, tag="wsrc")
        nc.gpsimd.memset(wsrc[:], 0.0)
        for w in range(N_WARM):
            wp = ps_st.tile([128, QB], F32, tag="st", name=f"warmmm{w}")
            nc.tensor.matmul(wp[:], ones_s[:], wsrc[:], start=True, stop=True)

        # ---- projections (just-in-time emission below for later chunks) ----
        def proj_mm(dst, w, src_slice):
            pt = ps_ms.tile([128, QB], F32, tag="pj", bufs=1)
            nc.tensor.matmul(pt[:], w, src_slice, start=True, stop=True)
            nc.vector.tensor_copy(dst, pt[:])

        proj_mm(QT_s[:, 0:QB], wq_s[:], xqT_s[:, 0:QB])
        proj_mm(KT_s[:, 0:QB], wk_s[:], xT_s[:, 0:QB])
        proj_mm(KT_s[:, QB:2 * QB], wk_s[:], xT_s[:, QB:2 * QB])
        kt_done = 2
        qt_done = 1

        # ---- attention: flat software pipeline over (qblock, group) ----
        ALL = [(qb, g) for qb in range(NQB) for g in range(NGRP)]

        def emit_st(qb, g):
            q0 = qb * QB
            st = ps_st.tile([128, GROUP * QB], F32, tag="st")
            for i in range(GROUP):
                kc = g * GROUP + i
                h = kc % 2  # row-tile half: concurrent 64-contraction pairs
                lhsT = KT_s[h * 64:(h + 1) * 64, kc * KC:(kc + 1) * KC]
                rhs = QT_s[h * 64:(h + 1) * 64, q0:q0 + QB]
                nc.tensor.matmul(st[:, i * QB:(i + 1) * QB], lhsT, rhs,
                                 start=True, stop=True)
            return st

        st_tiles = {}
        st_tiles[ALL[0]] = emit_st(*ALL[0])
        st_tiles[ALL[1]] = emit_st(*ALL[1])

        px_tiles = {}    # per-qblock PX^T [d, q] psum accumulators
        dps_tiles = {}   # per-qblock D psum accumulators (partition-broadcast)
        dwork = {}       # (qb, g) -> tile(s) for the lagged D-matmuls

        def emit_dmm(qb, g):
            """Accumulating ones-matmuls for group g of block qb."""
            kind, tiles = dwork.pop((qb, g))
            if qb not in dps_tiles:
                dps_tiles[qb] = ps_ms.tile([128, QB], F32, tag="dps",
                                           name=f"dps{qb}", bufs=1)
            dps = dps_tiles[qb]
            first = (g == 0)
            if kind == "t1":
                last = (g == NGRP - 1)
                nc.tensor.matmul(dps[:], ones_s[:], tiles[0][:],
                                 start=first, stop=last)
            else:  # raw p group tile, two chunk matmuls
                p = tiles[0]
                for i in range(GROUP):
                    last = (g == NGRP - 1) and (i == GROUP - 1)
                    nc.tensor.matmul(dps[:], ones_s[:],
                                     p[:, i * QB:(i + 1) * QB],
                                     start=(first and i == 0), stop=last)

        def finish_block(qb):
            q0 = qb * QB
            dinvb = mpool.tile([128, QB], F32, tag="dinvb")
            nc.vector.reciprocal_approx_fast(dinvb[:], dps_tiles.pop(qb)[:])
            pxn = mpool.tile([128, QB], BF16, tag="pxn")
            nc.vector.tensor_mul(pxn[:], px_tiles.pop(qb)[:], dinvb[:])
            po = ps_ms.tile([128, QB], F32, tag="pj", name=f"po{qb}", bufs=1)
            nc.tensor.matmul(po[:], wv_s[:], pxn[:], start=True, stop=True)
            ot = mpool.tile([128, QB], F32, tag="ot")
            nc.vector.tensor_copy(ot[:], po[:])
            nc.sync.dma_start(out[:, q0:q0 + QB], ot[:])

        for idx, (qb, g) in enumerate(ALL):
            st = st_tiles.pop((qb, g))
            p = ppool.tile([128, GROUP * QB], BF16, tag="p")
            if g % DVE_EXP_MOD == 1:
                # Schraudolph exp on DVE: p_bits = s*A + B, u16-converted
                nc.vector.tensor_scalar(
                    p[:].bitcast(U16), st[:], SCH_A, SCH_B,
                    mybir.AluOpType.mult, mybir.AluOpType.add)
            else:
                nc.scalar.activation(p[:], st[:], EXP, scale=0.125)

            if qb not in px_tiles:
                px_tiles[qb] = ps_px.tile([128, QB], F32, tag="px",
                                          name=f"px{qb}")
            px = px_tiles[qb]
            for i in range(GROUP):
                kc = g * GROUP + i
                nc.tensor.matmul(px[:], xc_s[:, kc * KC:(kc + 1) * KC],
                                 p[:, i * QB:(i + 1) * QB],
                                 start=(kc == 0), stop=(kc == NKC - 1))

            if idx + 2 < len(ALL):
                st_tiles[ALL[idx + 2]] = emit_st(*ALL[idx + 2])

            # D-reduction feed: pair-sum on DVE for the first TJ groups,
            # raw p chunks otherwise; actual D-matmuls lag by DMM_LAG
            if g < TJ:
                t1 = tpool.tile([128, QB], BF16, tag="t1")
                nc.vector.tensor_add(t1[:], p[:, 0:QB], p[:, QB:2 * QB])
                dwork[(qb, g)] = ("t1", [t1])
            else:
                dwork[(qb, g)] = ("raw", [p])
            lag_idx = idx - DMM_LAG
            if lag_idx >= 0:
                emit_dmm(*ALL[lag_idx])

            # just-in-time projections: KT chunk j feeds ST groups 2j..2j+1
            # (emitted 2 ahead), QT block j feeds q-block j
            need_kt = min(8, (idx + 4) // 2 + 1)
            while kt_done < need_kt:
                proj_mm(KT_s[:, kt_done * QB:(kt_done + 1) * QB], wk_s[:],
                        xT_s[:, kt_done * QB:(kt_done + 1) * QB])
                kt_done += 1
            need_qt = min(NQB, (idx + 4) // NGRP + 1)
            while qt_done < need_qt:
                proj_mm(QT_s[:, qt_done * QB:(qt_done + 1) * QB], wq_s[:],
                        xqT_s[:, qt_done * QB:(qt_done + 1) * QB])
                qt_done += 1

            if g == FINISH_DELAY - 1 and qb > 0:
                finish_block(qb - 1)

        for lag_idx in range(len(ALL) - DMM_LAG, len(ALL)):
            emit_dmm(*ALL[lag_idx])
        finish_block(NQB - 1)

    _log("build_nc: bacc compile")
    nc.compile()
    _log("build_nc: done")
    return nc


def _host_prep(x, Wq, Wk, Wv):
    x = np.asarray(x, dtype=np.float32)
    Wq = np.asarray(Wq, dtype=np.float32)
    Wk = np.asarray(Wk, dtype=np.float32)
    Wv = np.asarray(Wv, dtype=np.float32)
    wqTd = np.ascontiguousarray(
        np.concatenate([Wq.T, Wq.T], axis=1)).astype(NPBF16)
    wkTd = np.ascontiguousarray(
        np.concatenate([Wk.T, Wk.T], axis=1)).astype(NPBF16)
    wvT = np.ascontiguousarray(Wv.T).astype(NPBF16)
    in_maps = []
    for c in range(8):
        b, h = c // 2, c % 2
        xb = x[b]
        in_maps.append({
            "xT": np.ascontiguousarray(xb.T).astype(NPBF16),
            "xc": np.ascontiguousarray(
                xb.reshape(NKC, KC, D).transpose(1, 0, 2).reshape(KC, S)
            ).astype(NPBF16),
            "xqT": np.ascontiguousarray(
                xb[h * NQ:(h + 1) * NQ].T).astype(NPBF16),
            "wqTd": wqTd, "wkTd": wkTd, "wvT": wvT,
        })
    return in_maps


def run(x, Wq, Wk, Wv, trace=False, **kw):
    global _CACHED_NC
    if _CACHED_NC is None:
        _CACHED_NC = build_nc()
    in_maps = _host_prep(x, Wq, Wk, Wv)
    _log("run_bass_kernel_spmd (includes NEFF compile on first call)")
    res = run_bass_kernel_spmd(
        _CACHED_NC, in_maps, core_ids=list(range(8)), trace=trace, **kw)
    _log("run_bass_kernel_spmd returned")
    full = np.zeros((B, S, D), np.float32)
    for c in range(8):
        b, h = c // 2, c % 2
        full[b, h * NQ:(h + 1) * NQ] = np.asarray(
            res.results[c]["out"]).astype(np.float32).T
    return full, res


def kernel(x, Wq, Wk, Wv):
    full, _ = run(x, Wq, Wk, Wv, trace=False)
    return full


# revision 7
# speedup vs baseline: 1.3735x; 1.0559x over previous
"""Self-contained Trainium2 attention-block kernel (8 NeuronCores, SPMD).

Problem: x[4,4096,128], Wq/Wk[64,128], Wv[128,128] ->
  softmax((x Wq^T)(x Wk^T)^T / 8) (x Wv^T)   -> [4,4096,128] f32

Sharding: data-parallel over batch (4) x query-halves (2) = 8 cores.
Each core: q rows 2048, full K (4096) recomputed locally. No collectives.

v2 design (vs v1 which was ScalarE-bound at ~117us):
  - P*V reassociated as (P*X)*Wv^T: the per-chunk PV matmul uses raw x
    chunks as stationary weights (no V projection, no V casts); one
    [d,v] Wv matmul per q-block at the end; output DMA'd f32 from PSUM.
  - softmax denominator D via PE: accumulating ones-matmuls into a psum
    bank (rhs = bf16 pair-sums t1 for the first TJ groups, raw p chunks
    for the rest) instead of a deep DVE add-tree.
  - exp split across engines: every 4th group's exp runs on the DVE as
    a one-instruction Schraudolph bit-trick (u16 = s*A + B viewed as
    bf16 ~= exp(s/8)); the rest stay on ScalarE's table exp.
  - PE warm-up spin during the prologue so HAM reaches K=8/8 before
    steady state, plus weights-first DMA order and just-in-time
    projection emission.
"""

import sys

sys.path.insert(0, "/opt/trn_rl_repo")

from contextlib import ExitStack

import ml_dtypes
import numpy as np

import concourse.bass as bass  # noqa: F401
import concourse.bacc as bacc
import concourse.tile as tile
from concourse import mybir
from concourse.bass_utils import run_bass_kernel_spmd

BF16 = mybir.dt.bfloat16
F32 = mybir.dt.float32
U16 = mybir.dt.uint16
NPBF16 = ml_dtypes.bfloat16

B, S, D, A = 4, 4096, 128, 64
NQ = S // 2          # q rows per core
QB = 512             # q block (psum bank free size)
KC = 128             # k chunk (matmul contraction tile)
NKC = S // KC        # 32 chunks
NQB = NQ // QB       # 4 q blocks
GROUP = 2            # k chunks per exp group ([128,1024] psum tile)
NGRP = NKC // GROUP  # 16 groups per block
EXP = mybir.ActivationFunctionType.Exp

# tuning knobs
DVE_EXP_GROUPS = (1, 5, 9)  # groups per qblock whose exp runs on DVE (Schraudolph)
DMM_LAG = 2       # groups between a t2 tile's last input and its D-matmul
FINISH_DELAY = 3  # groups into the next block before finishing a block
N_WARM = 14       # prologue PE warm-up matmuls (N=512)

# Schraudolph constants: u16 = round(s * SCH_A + SCH_B) viewed as bf16
# approximates exp(s/8).  t = s*log2(e)/8; bits = 128*t + (127*128 - C).
SCH_A = 128 * np.log2(np.e) / 8          # 23.083120654223414
SCH_B = 16256.0 - 7.5                    # C=7.5 splits round/trunc modes

_CACHED_NC = None


def _log(msg):
    import time as _t
    print(f"[kernel {_t.strftime('%H:%M:%S')}] {msg}", file=sys.stderr, flush=True)


def build_nc():
    _log("build_nc: tracing graph")
    nc = bacc.Bacc(
        "TRN2", target_bir_lowering=False, debug=False,
        enable_asserts=False, num_devices=8,
    )
    xT = nc.dram_tensor("xT", [D, S], BF16, kind="ExternalInput").ap()
    xc = nc.dram_tensor("xc", [128, S], BF16, kind="ExternalInput").ap()
    xqT = nc.dram_tensor("xqT", [D, NQ], BF16, kind="ExternalInput").ap()
    wqTd = nc.dram_tensor("wqTd", [D, 128], BF16, kind="ExternalInput").ap()
    wkTd = nc.dram_tensor("wkTd", [D, 128], BF16, kind="ExternalInput").ap()
    wvT = nc.dram_tensor("wvT", [D, D], BF16, kind="ExternalInput").ap()
    # outT layout [v, q] f32; host transposes during gather
    out = nc.dram_tensor("out", [D, NQ], F32, kind="ExternalOutput").ap()

    with tile.TileContext(nc) as tc, ExitStack() as ctx:
        persist = ctx.enter_context(tc.tile_pool(name="persist", bufs=1))
        # PSUM: st 2x(2 banks) + px 2x(1 bank) + misc 4x(1 bank shared pairwise)
        ps_st = ctx.enter_context(tc.tile_pool(name="ps_st", bufs=2, space="PSUM"))
        ps_px = ctx.enter_context(tc.tile_pool(name="ps_px", bufs=2, space="PSUM"))
        ps_ms = ctx.enter_context(tc.tile_pool(name="ps_ms", bufs=1, space="PSUM"))
        ppool = ctx.enter_context(tc.tile_pool(name="ppool", bufs=5))
        tpool = ctx.enter_context(tc.tile_pool(name="tpool", bufs=6))
        mpool = ctx.enter_context(tc.tile_pool(name="mpool", bufs=4))

        # ---- persistent SBUF ----
        wq_s = persist.tile([D, 128], BF16, tag="wq_s")
        wk_s = persist.tile([D, 128], BF16, tag="wk_s")
        wv_s = persist.tile([D, D], BF16, tag="wv_s")
        ones_s = persist.tile([128, 128], BF16, tag="ones_s")
        xqT_s = persist.tile([D, NQ], BF16, tag="xqT_s")
        xT_s = persist.tile([D, S], BF16, tag="xT_s")
        xc_s = persist.tile([128, S], BF16, tag="xc_s")
        KT_s = persist.tile([128, S], BF16, tag="KT_s")   # duplicated halves
        QT_s = persist.tile([128, NQ], BF16, tag="QT_s")  # duplicated halves

        # ones for the D-matmuls needs no DMA
        nc.gpsimd.memset(ones_s[:], 1.0)

        # input DMAs: tiny weights first, then the chunks the first
        # projections need, then the rest
        nc.sync.dma_start(wq_s[:], wqTd[:])
        nc.sync.dma_start(wk_s[:], wkTd[:])
        nc.sync.dma_start(xqT_s[:, 0:QB], xqT[:, 0:QB])
        for j in range(4):
            nc.sync.dma_start(xT_s[:, j * 1024:(j + 1) * 1024],
                              xT[:, j * 1024:(j + 1) * 1024])
        for j in range(4):
            nc.sync.dma_start(xc_s[:, j * 1024:(j + 1) * 1024],
                              xc[:, j * 1024:(j + 1) * 1024])
        nc.sync.dma_start(xqT_s[:, QB:], xqT[:, QB:])
        nc.sync.dma_start(wv_s[:], wvT[:])

        # prewarm the exp table (ScalarE) off the critical path
        warm = persist.tile([1, 1], F32, tag="warm")
        nc.gpsimd.memset(warm[:], 1.0)
        warm2 = persist.tile([1, 1], F32, tag="warm2")
        nc.scalar.activation(warm2[:], warm[:], EXP)

        # PE warm-up spin: keep TensorE busy through the prologue so HAM
        # reaches K=8/8 before the real matmuls start
        wsrc = persist.tile([128, QB], BF16, tag="wsrc")
        nc.gpsimd.memset(wsrc[:], 0.0)
        for w in range(N_WARM):
            wp = ps_st.tile([128, QB], F32, tag="st", name=f"warmmm{w}")
            nc.tensor.matmul(wp[:], ones_s[:], wsrc[:], start=True, stop=True)

        # ---- projections (just-in-time emission below for later chunks) ----
        def proj_mm(dst, w, src_slice):
            pt = ps_ms.tile([128, QB], F32, tag="pj", bufs=1)
            nc.tensor.matmul(pt[:], w, src_slice, start=True, stop=True)
            nc.vector.tensor_copy(dst, pt[:])

        proj_mm(QT_s[:, 0:QB], wq_s[:], xqT_s[:, 0:QB])
        proj_mm(KT_s[:, 0:QB], wk_s[:], xT_s[:, 0:QB])
        proj_mm(KT_s[:, QB:2 * QB], wk_s[:], xT_s[:, QB:2 * QB])
        kt_done = 2
        qt_done = 1

        # ---- attention: flat software pipeline over (qblock, group) ----
        ALL = [(qb, g) for qb in range(NQB) for g in range(NGRP)]

        def emit_st(qb, g):
            q0 = qb * QB
            st = ps_st.tile([128, GROUP * QB], F32, tag="st")
            for i in range(GROUP):
                kc = g * GROUP + i
                h = kc % 2  # row-tile half: concurrent 64-contraction pairs
                lhsT = KT_s[h * 64:(h + 1) * 64, kc * KC:(kc + 1) * KC]
                rhs = QT_s[h * 64:(h + 1) * 64, q0:q0 + QB]
                nc.tensor.matmul(st[:, i * QB:(i + 1) * QB], lhsT, rhs,
                                 start=True, stop=True)
            return st

        st_tiles = {}
        st_tiles[ALL[0]] = emit_st(*ALL[0])
        st_tiles[ALL[1]] = emit_st(*ALL[1])

        px_tiles = {}    # per-qblock PX^T [d, q] psum accumulators
        dps_tiles = {}   # per-qblock D psum accumulators (partition-broadcast)
        t1_tiles = {}    # (qb, g) -> bf16 chunk-pair sums (DVE)
        pending = {}     # emission idx -> list of closures (lagged PE work)

        def emit_dmm(qb, j):
            """Accumulating ones-matmul for t2 tile j of block qb."""
            if qb not in dps_tiles:
                dps_tiles[qb] = ps_ms.tile([128, QB], F32, tag="dps",
                                           name=f"dps{qb}", bufs=1)
            t2 = tpool.tile([128, QB], BF16, tag="t2", bufs=4,
                            name=f"t2_{qb}_{j}")
            nc.gpsimd.tensor_add(t2[:], t1_tiles.pop((qb, 2 * j))[:],
                                 t1_tiles.pop((qb, 2 * j + 1))[:])
            dps = dps_tiles[qb]
            nc.tensor.matmul(dps[:], ones_s[:], t2[:],
                             start=(j == 0), stop=(j == NGRP // 2 - 1))

        def finish_block(qb):
            q0 = qb * QB
            dinvb = mpool.tile([128, QB], F32, tag="dinvb")
            nc.vector.reciprocal_approx_fast(dinvb[:], dps_tiles.pop(qb)[:])
            pxn = mpool.tile([128, QB], BF16, tag="pxn")
            nc.vector.tensor_mul(pxn[:], px_tiles.pop(qb)[:], dinvb[:])
            po = ps_ms.tile([128, QB], F32, tag="pj", name=f"po{qb}", bufs=1)
            nc.tensor.matmul(po[:], wv_s[:], pxn[:], start=True, stop=True)
            ot = mpool.tile([128, QB], F32, tag="ot")
            nc.vector.tensor_copy(ot[:], po[:])
            nc.sync.dma_start(out[:, q0:q0 + QB], ot[:])

        for idx, (qb, g) in enumerate(ALL):
            # lagged t2-adds (GpSimd) + D-matmuls (PE) scheduled for this slot
            for fn in pending.pop(idx, ()):
                fn()

            st = st_tiles.pop((qb, g))
            p = ppool.tile([128, GROUP * QB], BF16, tag="p")
            if g in DVE_EXP_GROUPS:
                # Schraudolph exp on DVE: p_bits = s*A + B, u16-converted
                nc.vector.tensor_scalar(
                    p[:].bitcast(U16), st[:], SCH_A, SCH_B,
                    mybir.AluOpType.mult, mybir.AluOpType.add)
            else:
                nc.scalar.activation(p[:], st[:], EXP, scale=0.125)

            if idx + 2 < len(ALL):
                st_tiles[ALL[idx + 2]] = emit_st(*ALL[idx + 2])

            if qb not in px_tiles:
                px_tiles[qb] = ps_px.tile([128, QB], F32, tag="px",
                                          name=f"px{qb}")
            px = px_tiles[qb]
            for i in range(GROUP):
                kc = g * GROUP + i
                nc.tensor.matmul(px[:], xc_s[:, kc * KC:(kc + 1) * KC],
                                 p[:, i * QB:(i + 1) * QB],
                                 start=(kc == 0), stop=(kc == NKC - 1))

            # level-1 chunk-pair sum on DVE; level-2 + D-matmul lag behind
            t1 = tpool.tile([128, QB], BF16, tag="t1")
            nc.vector.tensor_add(t1[:], p[:, 0:QB], p[:, QB:2 * QB])
            t1_tiles[(qb, g)] = t1
            if g % 2 == 1:
                j = g // 2
                pending.setdefault(idx + DMM_LAG, []).append(
                    lambda qb=qb, j=j: emit_dmm(qb, j))

            # just-in-time projections: KT chunk j feeds ST groups 2j..2j+1
            # (emitted 2 ahead), QT block j feeds q-block j
            need_kt = min(8, (idx + 4) // 2 + 1)
            while kt_done < need_kt:
                proj_mm(KT_s[:, kt_done * QB:(kt_done + 1) * QB], wk_s[:],
                        xT_s[:, kt_done * QB:(kt_done + 1) * QB])
                kt_done += 1
            need_qt = min(NQB, (idx + 4) // NGRP + 1)
            while qt_done < need_qt:
                proj_mm(QT_s[:, qt_done * QB:(qt_done + 1) * QB], wq_s[:],
                        xqT_s[:, qt_done * QB:(qt_done + 1) * QB])
                qt_done += 1

            if g == FINISH_DELAY - 1 and qb > 0:
                finish_block(qb - 1)

        for idx in sorted(k for k in pending if k >= len(ALL)):
            for fn in pending.pop(idx):
                fn()
        finish_block(NQB - 1)

    _log("build_nc: bacc compile")
    nc.compile()
    _log("build_nc: done")
    return nc


def _host_prep(x, Wq, Wk, Wv):
    x = np.asarray(x, dtype=np.float32)
    Wq = np.asarray(Wq, dtype=np.float32)
    Wk = np.asarray(Wk, dtype=np.float32)
    Wv = np.asarray(Wv, dtype=np.float32)
    wqTd = np.ascontiguousarray(
        np.concatenate([Wq.T, Wq.T], axis=1)).astype(NPBF16)
    wkTd = np.ascontiguousarray(
        np.concatenate([Wk.T, Wk.T], axis=1)).astype(NPBF16)
    wvT = np.ascontiguousarray(Wv.T).astype(NPBF16)
    in_maps = []
    for c in range(8):
        b, h = c // 2, c % 2
        xb = x[b]
        in_maps.append({
            "xT": np.ascontiguousarray(xb.T).astype(NPBF16),
            "xc": np.ascontiguousarray(
                xb.reshape(NKC, KC, D).transpose(1, 0, 2).reshape(KC, S)
            ).astype(NPBF16),
            "xqT": np.ascontiguousarray(
                xb[h * NQ:(h + 1) * NQ].T).astype(NPBF16),
            "wqTd": wqTd, "wkTd": wkTd, "wvT": wvT,
        })
    return in_maps


def run(x, Wq, Wk, Wv, trace=False, **kw):
    global _CACHED_NC
    if _CACHED_NC is None:
        _CACHED_NC = build_nc()
    in_maps = _host_prep(x, Wq, Wk, Wv)
    _log("run_bass_kernel_spmd (includes NEFF compile on first call)")
    res = run_bass_kernel_spmd(
        _CACHED_NC, in_maps, core_ids=list(range(8)), trace=trace, **kw)
    _log("run_bass_kernel_spmd returned")
    full = np.zeros((B, S, D), np.float32)
    for c in range(8):
        b, h = c // 2, c % 2
        full[b, h * NQ:(h + 1) * NQ] = np.asarray(
            res.results[c]["out"]).astype(np.float32).T
    return full, res


def kernel(x, Wq, Wk, Wv):
    full, _ = run(x, Wq, Wk, Wv, trace=False)
    return full
